# revision 15
# baseline (speedup 1.0000x reference)
"""Child-Sum Tree-LSTM (reference.py nn_ChildSumTreeLSTM) on 8 Trainium2
NeuronCores via Bass/Tile, SPMD.

Strategy: everything transposed (features on SBUF partitions, nodes on the
free dimension). Each core owns a contiguous slice of every level (levels
cut..8); since children of a node are contiguous, the leaves->level-cut
recursion is fully core-local (no collectives). The top levels (cut-1..0,
few nodes) are finished on the host in numpy during the gather step.

Key scheduling ideas vs the naive version:
- All PSUM flows through one tag of [P,1024]-fp32 (2-bank) tiles with a
  4-deep rotation (8 banks total): one tile per (gate, feature-tile) or
  f-gate quarter, so TensorE can run ~3 tiles ahead of ScalarE.
- Leaf chunks of 1024 nodes give FD=1024 activation instructions (ScalarE
  has no perf modes, so its ~180-cycle per-instruction overhead is the
  only reducible part of its cost); emission interleaves ACT-heavy leaf
  chunks with tensor-heavy level-(D-1) f-gate/iou slices so the in-order
  per-engine queues always have runnable work.
- Gate order i,u,o: c = i*u and tanh(c) chase the u-tanh while the
  o-sigmoids keep ScalarE busy.
- Child sums and the f*c group reduction run as 2x-mode tensor_tensor add
  trees on DVE (tensor_reduce only has a 1x uop); leaf child-sums alternate
  DVE/GpSimd to keep both off the critical path.
- Matmuls in bf16 (fp32 PSUM), biases ride the activation instructions.
"""
import sys
sys.path.insert(0, '/opt/trn_rl_repo')
import numpy as np
import ml_dtypes
import concourse.bacc as bacc
import concourse.mybir as mybir
from concourse.tile import TileContext
from concourse.alu_op_type import AluOpType

F32 = mybir.dt.float32
BF16 = mybir.dt.bfloat16
AFT = mybir.ActivationFunctionType
P = 128
NCORES = 8
BR = 4


def level_offs(D):
    return [(BR ** l - 1) // (BR - 1) for l in range(D + 1)]


def local_counts(D, cut):
    return {l: BR ** l // NCORES for l in range(cut, D + 1)}


def local_offs(D, cut):
    n = local_counts(D, cut)
    offs = {}
    acc = 0
    for l in range(cut, D + 1):
        offs[l] = acc
        acc += n[l]
    return offs, acc


def build_program(D, cut, c_dtype=BF16):
    """Leaf chunks of SL=1024 nodes; level-7 parents in blocks of SP=512,
    f-gate work sliced per child chunk so everything pipelines.  All PSUM
    flows through one 4-bank tag with bufs=2 (8 banks total)."""
    nloc = local_counts(D, cut)
    loff, total_rows = local_offs(D, cut)
    CDT = c_dtype
    SL = 1024                     # leaf chunk (nodes)
    SP = 512                      # internal chunk/block (parents)

    nc = bacc.Bacc("TRN2", target_bir_lowering=False, debug=False,
                   num_devices=NCORES)
    xT = nc.dram_tensor("xT", [2, P, total_rows], BF16, kind="ExternalInput")
    wx = nc.dram_tensor("wx", [2, P, 1024], BF16, kind="ExternalInput")
    wh = nc.dram_tensor("wh", [2, P, 1024], BF16, kind="ExternalInput")
    bias = nc.dram_tensor("bias", [P, 8], F32, kind="ExternalInput")
    ncut = nloc[cut]
    out_h = nc.dram_tensor("out_h", [2, P, ncut], BF16, kind="ExternalOutput")
    out_c = nc.dram_tensor("out_c", [2, P, ncut], CDT, kind="ExternalOutput")

    with TileContext(nc) as tc:
        with tc.tile_pool(name="const", bufs=1) as constp, \
             tc.tile_pool(name="xin", bufs=2) as xin, \
             tc.tile_pool(name="state", bufs=1) as statep, \
             tc.tile_pool(name="work", bufs=3) as work, \
             tc.tile_pool(name="psum", bufs=1, space="PSUM") as psum:

            wxt = constp.tile([P, 2, 1024], BF16)
            wht = constp.tile([P, 2, 1024], BF16)
            bt = constp.tile([P, 8], F32)
            # iou weights first so the first leaf matmuls start ASAP
            nc.sync.dma_start(bt[:], bias[:])
            nc.sync.dma_start(wxt[:, :, 0:768],
                              wx[:, :, 0:768].rearrange("a p n -> p a n"))
            nc.sync.dma_start(wxt[:, :, 768:1024],
                              wx[:, :, 768:1024].rearrange("a p n -> p a n"))
            nc.sync.dma_start(wht[:], wh[:].rearrange("a p n -> p a n"))

            def load_x(l, c0, Sx, tag="xt", bufs=2):
                t = xin.tile([P, 2, Sx], BF16, tag=tag, bufs=bufs, name=tag)
                src = xT[:, :, loff[l] + c0: loff[l] + c0 + Sx]
                nc.sync.dma_start(t[:], src.rearrange("a p n -> p a n"))
                return t

            # ---- persistent level tiles (levels cut..D-1) ----
            lt_h = {}
            lt_c = {}
            hs_t = {}
            for l in range(cut, D):
                lt_h[l] = statep.tile([P, 2, nloc[l]], BF16, tag=f"h{l}",
                                      name=f"h{l}")
                lt_c[l] = statep.tile([P, 2, nloc[l]], CDT, tag=f"c{l}",
                                      name=f"c{l}")
                hs_t[l] = statep.tile([P, 2, nloc[l]], BF16, tag=f"hs{l}",
                                      name=f"hs{l}")

            def emit_hsum(lpar, ch_ap, c0p, Sp, eng):
                """Sum 4-child groups of ch_ap ([P,2,4*Sp]) into
                hs_t[lpar][:, :, c0p:c0p+Sp] with a 2-level add tree."""
                with nc.allow_low_precision(reason="bf16 by design"):
                    htmp = work.tile([P, 2, Sp, 2], BF16, tag="htmp",
                                     bufs=2, name="htmp")
                    for ft in range(2):
                        v = ch_ap[:, ft, :].rearrange("p (n b) -> p n b", b=BR)
                        eng.tensor_tensor(htmp[:, ft, :, :],
                                          v[:, :, 0:2], v[:, :, 2:4],
                                          AluOpType.add)
                        eng.tensor_tensor(hs_t[lpar][:, ft, c0p:c0p + Sp],
                                          htmp[:, ft, :, 0],
                                          htmp[:, ft, :, 1],
                                          AluOpType.add)

            def iou_gate_mms(xt, Sx, gidx, ft, hs=None):
                """One (gate, ftile) psum tile [P, Sx] (2-bank slot, 4-way
                rotation), filled by 512-wide matmul dsts."""
                mt = gidx * 2 + ft
                ps = psum.tile([P, Sx], F32, tag="g", bufs=4, name="ps")
                for q in range(max(1, Sx // 512)):
                    w_ = min(512, Sx)
                    dst = ps[:, q * 512:q * 512 + w_]
                    xs = slice(q * 512, q * 512 + w_)
                    nc.tensor.matmul(dst, wxt[:, 0, mt * P:(mt + 1) * P],
                                     xt[:, 0, xs], start=True, stop=False)
                    nc.tensor.matmul(dst, wxt[:, 1, mt * P:(mt + 1) * P],
                                     xt[:, 1, xs], start=False,
                                     stop=hs is None)
                    if hs is not None:
                        nc.tensor.matmul(dst, wht[:, 0, mt * P:(mt + 1) * P],
                                         hs[:, 0, xs], start=False,
                                         stop=False)
                        nc.tensor.matmul(dst, wht[:, 1, mt * P:(mt + 1) * P],
                                         hs[:, 1, xs], start=False,
                                         stop=True)
                return ps

            def iou_gates(xt, Sx, hs=None):
                """Gate order i,u,o so c=i*u can chase the u-tanh while the
                o-sigmoids keep ScalarE busy.  Each (gate, ft) is one psum
                tile + one FD=Sx activation."""
                it = work.tile([P, 2, Sx], BF16, tag="it", name="it")
                ot = work.tile([P, 2, Sx], BF16, tag="ot", name="ot")
                ut = work.tile([P, 2, Sx], BF16, tag="ut", name="ut")
                for gidx, dst, fn in ((0, it, AFT.Sigmoid),
                                      (2, ut, AFT.Tanh),
                                      (1, ot, AFT.Sigmoid)):
                    for ft in range(2):
                        mt = gidx * 2 + ft
                        ps = iou_gate_mms(xt, Sx, gidx, ft, hs)
                        nc.scalar.activation(dst[:, ft, :], ps[:], fn,
                                             bias=bt[:, mt:mt + 1])
                return it, ot, ut

            lh = {}                # leaf chunk h/c tiles, by chunk index
            lc = {}

            def leaf_chunk(k, off, w, hsum_eng):
                xt = load_x(D, off, w, tag="xleaf", bufs=3)
                it, ot, ut = iou_gates(xt, w)
                lh[k] = work.tile([P, 2, w], BF16, tag="lh", bufs=5,
                                  name="lh", padded_shape=[P, 2, 1024])
                lc[k] = work.tile([P, 2, w], CDT, tag="lc", bufs=5,
                                  name="lc", padded_shape=[P, 2, 1024])
                with nc.allow_low_precision(reason="bf16 by design"):
                    nc.vector.tensor_tensor(lc[k][:], it[:], ut[:],
                                            AluOpType.mult)
                    nc.scalar.activation(ut[:, :, 0:w], lc[k][:], AFT.Tanh)
                    nc.vector.tensor_tensor(lh[k][:], ot[:], ut[:, :, 0:w],
                                            AluOpType.mult)
                emit_hsum(D - 1, lh[k][:], off // BR, w // BR, hsum_eng)

            # ---- internal blocks ----
            def make_state(lv, c0, Sp):
                """Block of Sp parents at parent offset c0 of level lv.
                st["ch"] collects (h_ap, c_ap, child_off, width) entries."""
                xt = load_x(lv, c0, Sp, tag="xi", bufs=3)
                fcs = work.tile([P, 2, Sp], BF16, tag="fcs", bufs=3,
                                name="fcs", padded_shape=[P, 2, SP])
                return {"lv": lv, "c0": c0, "Sp": Sp, "xt": xt, "ch": [],
                        "fcs": fcs, "hs": hs_t[lv][:, :, c0:c0 + Sp],
                        "h_dst": lt_h[lv][:, :, c0:c0 + Sp],
                        "c_dst": lt_c[lv][:, :, c0:c0 + Sp]}

            def int_f_ent(st, ent, ftt):
                """f-gates for the parents whose children live in one child
                chunk (ent), one feature tile."""
                ch_h, ch_c, coff, nq = ent
                Sq = nq // BR                 # parents covered
                p0 = coff // BR               # their offset within the block
                xt = st["xt"]
                woff = 768 + ftt * P
                pf = psum.tile([P, nq], F32, tag="g", bufs=4, name="pf",
                               padded_shape=[P, 1024])
                for q in range(max(1, nq // 512)):
                    w_ = min(512, nq)
                    lo = q * 512
                    dst = pf[:, lo:lo + w_]
                    nc.tensor.matmul(dst, wht[:, 0, woff:woff + P],
                                     ch_h[:, 0, lo:lo + w_],
                                     start=True, stop=False)
                    nc.tensor.matmul(dst, wht[:, 1, woff:woff + P],
                                     ch_h[:, 1, lo:lo + w_],
                                     start=False, stop=False)
                    plo, pw = p0 + lo // BR, w_ // BR
                    for kt in range(2):
                        rhs = xt[:, kt, plo:plo + pw] \
                            .rearrange("p (n b) -> p n b", b=1) \
                            .broadcast_to([P, pw, BR])
                        nc.tensor.matmul(
                            dst.rearrange("p (n b) -> p n b", b=BR),
                            wxt[:, kt, woff:woff + P],
                            rhs, start=False, stop=(kt == 1))
                fq = work.tile([P, nq], BF16, tag="fq", bufs=2, name="fq",
                               padded_shape=[P, 1024])
                nc.scalar.activation(fq[:], pf[:], AFT.Sigmoid,
                                     bias=bt[:, 6 + ftt:7 + ftt])
                with nc.allow_low_precision(reason="bf16 by design"):
                    nc.vector.tensor_tensor(fq[:], fq[:], ch_c[:, ftt, :],
                                            AluOpType.mult)
                    v = fq[:].rearrange("p (n b) -> p n b", b=BR)
                    ftmp = work.tile([P, Sq, 2], BF16, tag="ftmp", bufs=2,
                                     name="ftmp", padded_shape=[P, 256, 2])
                    nc.vector.tensor_tensor(ftmp[:], v[:, :, 0:2],
                                            v[:, :, 2:4], AluOpType.add)
                    nc.vector.tensor_tensor(
                        st["fcs"][:, ftt, p0:p0 + Sq],
                        ftmp[:, :, 0], ftmp[:, :, 1], AluOpType.add)

            def int_iou_fin(st):
                it, ot, ut = iou_gates(st["xt"], st["Sp"], st["hs"])
                fcs = st["fcs"]
                lv = st["lv"]
                with nc.allow_low_precision(reason="bf16 by design"):
                    nc.vector.tensor_tensor(it[:], it[:], ut[:],
                                            AluOpType.mult)
                    nc.vector.tensor_tensor(st["c_dst"], it[:], fcs[:],
                                            AluOpType.add)
                    nc.scalar.activation(ut[:], st["c_dst"], AFT.Tanh)
                    nc.vector.tensor_tensor(st["h_dst"], ot[:], ut[:],
                                            AluOpType.mult)
                if lv - 1 >= cut:
                    emit_hsum(lv - 1, st["h_dst"], st["c0"] // BR,
                              st["Sp"] // BR, nc.vector)
                if lv == cut:
                    c0, Sp = st["c0"], st["Sp"]
                    nc.sync.dma_start(
                        out_h[:, :, c0:c0 + Sp].rearrange("a p n -> p a n"),
                        st["h_dst"])
                    nc.sync.dma_start(
                        out_c[:, :, c0:c0 + Sp].rearrange("a p n -> p a n"),
                        st["c_dst"])

            # ---- schedule: leaves + level D-1 blocks, pipelined ----
            # First two leaf chunks are halved so the ScalarE pipeline fills
            # sooner; the last two parent blocks are halved so the final
            # (un-overlapped) f+iou chain is short.
            lp = D - 1
            widths = [512, 512] + [1024] * 7
            choffs = [0, 512] + [1024 * (i + 1) for i in range(7)]
            blist = [(0, 512, (0, 1, 2)), (512, 512, (3, 4)),
                     (1024, 512, (5, 6)), (1536, 256, (7,)),
                     (1792, 256, (8,))]
            owner = {}
            for bi, (_, _, cks) in enumerate(blist):
                for ci in cks:
                    owner[ci] = bi
            bstate = {}
            todo = []          # (fn, args) queue of internal slices
            for k in range(len(widths)):
                leaf_chunk(k, choffs[k], widths[k],
                           nc.gpsimd if k % 2 else nc.vector)
                # drain pending internal slices between leaf chunks
                emitted = 0
                while todo and emitted < 3:
                    fn, a = todo.pop(0)
                    fn(*a)
                    emitted += 1
                bi = owner[k]
                poff, pcnt, cks = blist[bi]
                if bi not in bstate:
                    bstate[bi] = make_state(lp, poff, pcnt)
                st = bstate[bi]
                ent = (lh[k][:], lc[k][:], choffs[k] - poff * BR, widths[k])
                st["ch"].append(ent)
                todo += [(int_f_ent, (st, ent, 0)),
                         (int_f_ent, (st, ent, 1))]
                if k == cks[-1]:
                    todo.append((int_iou_fin, (st,)))
            for fn, a in todo:
                fn(*a)

            # ---- levels D-2 .. cut (children from lt tiles) ----
            for lv in range(D - 2, cut - 1, -1):
                Sp = min(SP, nloc[lv])
                for c0 in range(0, nloc[lv], Sp):
                    nch = BR * Sp
                    st = make_state(lv, c0, Sp)
                    hw = nch // 2
                    for i in range(2):
                        ent = (lt_h[lv + 1][:, :, c0 * BR + i * hw:
                                            c0 * BR + (i + 1) * hw],
                               lt_c[lv + 1][:, :, c0 * BR + i * hw:
                                            c0 * BR + (i + 1) * hw],
                               i * hw, hw)
                        st["ch"].append(ent)
                        for ftt in range(2):
                            int_f_ent(st, ent, ftt)
                    int_iou_fin(st)

    nc.compile()
    return nc


def shard_inputs(x, W_iou_x, b_iou_x, W_iou_h, b_iou_h, W_fx, b_fx, W_fh, b_fh,
                 D, cut):
    offs = level_offs(D)
    nloc = local_counts(D, cut)
    wx_cat = np.concatenate([W_iou_x, W_fx], axis=0)
    wh_cat = np.concatenate([W_iou_h, W_fh], axis=0)
    wx_d = np.ascontiguousarray(wx_cat.T).reshape(2, P, 1024).astype(ml_dtypes.bfloat16)
    wh_d = np.ascontiguousarray(wh_cat.T).reshape(2, P, 1024).astype(ml_dtypes.bfloat16)
    b_iou = (b_iou_x + b_iou_h).reshape(6, P).T
    b_f = (b_fx + b_fh).reshape(2, P).T
    bias = np.ascontiguousarray(
        np.concatenate([b_iou, b_f], axis=1)).astype(np.float32)
    in_maps = []
    for k in range(NCORES):
        rows = []
        for l in range(cut, D + 1):
            n = nloc[l]
            rows.append(x[offs[l] + k * n: offs[l] + (k + 1) * n])
        xl = np.concatenate(rows, axis=0)
        xTk = np.ascontiguousarray(xl.T).reshape(2, P, -1).astype(ml_dtypes.bfloat16)
        in_maps.append({"xT": xTk, "wx": wx_d, "wh": wh_d, "bias": bias})
    return in_maps


def finish_host(results, x, W_iou_x, b_iou_x, W_iou_h, b_iou_h,
                W_fx, b_fx, W_fh, b_fh, D, cut):
    ncut = BR ** cut
    npc = ncut // NCORES
    Hc = np.empty((ncut, 256), np.float32)
    Cc = np.empty((ncut, 256), np.float32)
    for k in range(NCORES):
        oh = results[k]["out_h"].astype(np.float32).reshape(256, npc)
        oc = results[k]["out_c"].astype(np.float32).reshape(256, npc)
        Hc[k * npc:(k + 1) * npc] = oh.T
        Cc[k * npc:(k + 1) * npc] = oc.T
    sig = lambda v: 1.0 / (1.0 + np.exp(-v))
    h_next, c_next = Hc, Cc
    for l in range(cut - 1, -1, -1):
        n, off = BR ** l, (BR ** l - 1) // 3
        xl = x[off:off + n]
        child_h = h_next.reshape(n, BR, 256)
        child_c = c_next.reshape(n, BR, 256)
        chs = child_h.sum(axis=1)
        iou = xl @ W_iou_x.T + b_iou_x + chs @ W_iou_h.T + b_iou_h
        i, o, u = np.split(iou, 3, axis=1)
        i, o, u = sig(i), sig(o), np.tanh(u)
        f = sig(child_h @ W_fh.T + b_fh + (xl @ W_fx.T + b_fx)[:, None, :])
        c = i * u + (f * child_c).sum(axis=1)
        h = o * np.tanh(c)
        h_next, c_next = h, c
    return c_next.astype(np.float32), h_next.astype(np.float32)


# ---------------- public API ----------------

_D = 8
_CUT = 7
_CACHE = {}


def _get_program():
    if "nc" not in _CACHE:
        _CACHE["nc"] = build_program(_D, _CUT)
    return _CACHE["nc"]


def kernel(x, W_iou_x, b_iou_x, W_iou_h, b_iou_h, W_fx, b_fx, W_fh, b_fh):
    from concourse import bass_utils
    x = np.asarray(x, dtype=np.float32)
    args = [np.asarray(a, dtype=np.float32) for a in
            (W_iou_x, b_iou_x, W_iou_h, b_iou_h, W_fx, b_fx, W_fh, b_fh)]
    nc = _get_program()
    in_maps = shard_inputs(x, *args, _D, _CUT)
    res = bass_utils.run_bass_kernel_spmd(nc, in_maps,
                                          core_ids=list(range(NCORES)))
    c, h = finish_host(res.results, x, *args, _D, _CUT)
    return c, h


# revision 16
# speedup vs baseline: 1.0207x; 1.0207x over previous
"""Child-Sum Tree-LSTM (reference.py nn_ChildSumTreeLSTM) on 8 Trainium2
NeuronCores via Bass/Tile, SPMD.

Strategy: everything transposed (features on SBUF partitions, nodes on the
free dimension). Each core owns a contiguous slice of every level (levels
cut..8); since children of a node are contiguous, the leaves->level-cut
recursion is fully core-local (no collectives). The top levels (cut-1..0,
few nodes) are finished on the host in numpy during the gather step.

Key scheduling ideas vs the naive version:
- All PSUM flows through one tag of [P,1024]-fp32 (2-bank) tiles with a
  4-deep rotation (8 banks total): one tile per (gate, feature-tile) or
  f-gate quarter, so TensorE can run ~3 tiles ahead of ScalarE.
- Leaf chunks of 1024 nodes give FD=1024 activation instructions (ScalarE
  has no perf modes, so its ~180-cycle per-instruction overhead is the
  only reducible part of its cost); emission interleaves ACT-heavy leaf
  chunks with tensor-heavy level-(D-1) f-gate/iou slices so the in-order
  per-engine queues always have runnable work.
- Gate order i,u,o: c = i*u and tanh(c) chase the u-tanh while the
  o-sigmoids keep ScalarE busy.
- Child sums and the f*c group reduction run as 2x-mode tensor_tensor add
  trees on DVE (tensor_reduce only has a 1x uop); leaf child-sums alternate
  DVE/GpSimd to keep both off the critical path.
- Matmuls in bf16 (fp32 PSUM), biases ride the activation instructions.
"""
import sys
sys.path.insert(0, '/opt/trn_rl_repo')
import numpy as np
import ml_dtypes
import concourse.bacc as bacc
import concourse.mybir as mybir
from concourse.tile import TileContext
from concourse.alu_op_type import AluOpType

F32 = mybir.dt.float32
BF16 = mybir.dt.bfloat16
AFT = mybir.ActivationFunctionType
P = 128
NCORES = 8
BR = 4


def level_offs(D):
    return [(BR ** l - 1) // (BR - 1) for l in range(D + 1)]


def local_counts(D, cut):
    return {l: BR ** l // NCORES for l in range(cut, D + 1)}


def local_offs(D, cut):
    n = local_counts(D, cut)
    offs = {}
    acc = 0
    for l in range(cut, D + 1):
        offs[l] = acc
        acc += n[l]
    return offs, acc


def build_program(D, cut, c_dtype=BF16):
    """Leaf chunks of SL=1024 nodes; level-7 parents in blocks of SP=512,
    f-gate work sliced per child chunk so everything pipelines.  All PSUM
    flows through one 4-bank tag with bufs=2 (8 banks total)."""
    nloc = local_counts(D, cut)
    loff, total_rows = local_offs(D, cut)
    CDT = c_dtype
    SL = 1024                     # leaf chunk (nodes)
    SP = 512                      # internal chunk/block (parents)

    nc = bacc.Bacc("TRN2", target_bir_lowering=False, debug=False,
                   num_devices=NCORES)
    xT = nc.dram_tensor("xT", [2, P, total_rows], BF16, kind="ExternalInput")
    wx = nc.dram_tensor("wx", [2, P, 1024], BF16, kind="ExternalInput")
    wh = nc.dram_tensor("wh", [2, P, 1024], BF16, kind="ExternalInput")
    bias = nc.dram_tensor("bias", [P, 8], F32, kind="ExternalInput")
    ncut = nloc[cut]
    out_h = nc.dram_tensor("out_h", [2, P, ncut], BF16, kind="ExternalOutput")
    out_c = nc.dram_tensor("out_c", [2, P, ncut], CDT, kind="ExternalOutput")

    with TileContext(nc) as tc:
        with tc.tile_pool(name="const", bufs=1) as constp, \
             tc.tile_pool(name="xin", bufs=2) as xin, \
             tc.tile_pool(name="state", bufs=1) as statep, \
             tc.tile_pool(name="work", bufs=3) as work, \
             tc.tile_pool(name="psum", bufs=1, space="PSUM") as psum:

            wxt = constp.tile([P, 2, 1024], BF16)
            wht = constp.tile([P, 2, 1024], BF16)
            bt = constp.tile([P, 8], F32)
            scrap = constp.tile([P, 1], F32)
            # weights in first-use order so the first leaf matmuls (iou) and
            # first f-gates start as soon as possible
            nc.sync.dma_start(bt[:], bias[:])
            nc.sync.dma_start(wxt[:, :, 0:768],
                              wx[:, :, 0:768].rearrange("a p n -> p a n"))
            nc.sync.dma_start(wxt[:, :, 768:1024],
                              wx[:, :, 768:1024].rearrange("a p n -> p a n"))
            nc.sync.dma_start(wht[:, :, 768:1024],
                              wh[:, :, 768:1024].rearrange("a p n -> p a n"))
            nc.sync.dma_start(wht[:, :, 0:768],
                              wh[:, :, 0:768].rearrange("a p n -> p a n"))
            # dummy activation: pulls the ~1.5us ACT_TABLE_LOAD into the
            # initial DMA wait instead of serializing it with real work
            nc.scalar.activation(scrap[:], bt[:, 0:1], AFT.Sigmoid)

            def load_x(l, c0, Sx, tag="xt", bufs=2):
                t = xin.tile([P, 2, Sx], BF16, tag=tag, bufs=bufs, name=tag)
                src = xT[:, :, loff[l] + c0: loff[l] + c0 + Sx]
                nc.sync.dma_start(t[:], src.rearrange("a p n -> p a n"))
                return t

            # ---- persistent level tiles (levels cut..D-1) ----
            lt_h = {}
            lt_c = {}
            hs_t = {}
            for l in range(cut, D):
                lt_h[l] = statep.tile([P, 2, nloc[l]], BF16, tag=f"h{l}",
                                      name=f"h{l}")
                lt_c[l] = statep.tile([P, 2, nloc[l]], CDT, tag=f"c{l}",
                                      name=f"c{l}")
                hs_t[l] = statep.tile([P, 2, nloc[l]], BF16, tag=f"hs{l}",
                                      name=f"hs{l}")

            def emit_hsum(lpar, ch_ap, c0p, Sp, eng):
                """Sum 4-child groups of ch_ap ([P,2,4*Sp]) into
                hs_t[lpar][:, :, c0p:c0p+Sp] with a 2-level add tree."""
                with nc.allow_low_precision(reason="bf16 by design"):
                    htmp = work.tile([P, 2, Sp, 2], BF16, tag="htmp",
                                     bufs=2, name="htmp")
                    for ft in range(2):
                        v = ch_ap[:, ft, :].rearrange("p (n b) -> p n b", b=BR)
                        eng.tensor_tensor(htmp[:, ft, :, :],
                                          v[:, :, 0:2], v[:, :, 2:4],
                                          AluOpType.add)
                        eng.tensor_tensor(hs_t[lpar][:, ft, c0p:c0p + Sp],
                                          htmp[:, ft, :, 0],
                                          htmp[:, ft, :, 1],
                                          AluOpType.add)

            def iou_gate_mms(xt, Sx, gidx, ft, hs=None):
                """One (gate, ftile) psum tile [P, Sx] (2-bank slot, 4-way
                rotation), filled by 512-wide matmul dsts."""
                mt = gidx * 2 + ft
                ps = psum.tile([P, Sx], F32, tag="g", bufs=4, name="ps")
                for q in range(max(1, Sx // 512)):
                    w_ = min(512, Sx)
                    dst = ps[:, q * 512:q * 512 + w_]
                    xs = slice(q * 512, q * 512 + w_)
                    nc.tensor.matmul(dst, wxt[:, 0, mt * P:(mt + 1) * P],
                                     xt[:, 0, xs], start=True, stop=False)
                    nc.tensor.matmul(dst, wxt[:, 1, mt * P:(mt + 1) * P],
                                     xt[:, 1, xs], start=False,
                                     stop=hs is None)
                    if hs is not None:
                        nc.tensor.matmul(dst, wht[:, 0, mt * P:(mt + 1) * P],
                                         hs[:, 0, xs], start=False,
                                         stop=False)
                        nc.tensor.matmul(dst, wht[:, 1, mt * P:(mt + 1) * P],
                                         hs[:, 1, xs], start=False,
                                         stop=True)
                return ps

            def iou_gates(xt, Sx, hs=None):
                """Gate order i,u,o so c=i*u can chase the u-tanh while the
                o-sigmoids keep ScalarE busy.  Each (gate, ft) is one psum
                tile + one FD=Sx activation."""
                it = work.tile([P, 2, Sx], BF16, tag="it", name="it")
                ot = work.tile([P, 2, Sx], BF16, tag="ot", name="ot")
                ut = work.tile([P, 2, Sx], BF16, tag="ut", name="ut")
                for gidx, dst, fn in ((0, it, AFT.Sigmoid),
                                      (2, ut, AFT.Tanh),
                                      (1, ot, AFT.Sigmoid)):
                    for ft in range(2):
                        mt = gidx * 2 + ft
                        ps = iou_gate_mms(xt, Sx, gidx, ft, hs)
                        nc.scalar.activation(dst[:, ft, :], ps[:], fn,
                                             bias=bt[:, mt:mt + 1])
                return it, ot, ut

            lh = {}                # leaf chunk h/c tiles, by chunk index
            lc = {}

            def leaf_chunk(k, hsum_eng):
                xt = load_x(D, k * SL, SL, tag="xleaf", bufs=3)
                it, ot, ut = iou_gates(xt, SL)
                lh[k] = work.tile([P, 2, SL], BF16, tag="lh", bufs=3,
                                  name="lh")
                lc[k] = work.tile([P, 2, SL], CDT, tag="lc", bufs=3,
                                  name="lc")
                with nc.allow_low_precision(reason="bf16 by design"):
                    nc.vector.tensor_tensor(lc[k][:], it[:], ut[:],
                                            AluOpType.mult)
                    nc.scalar.activation(ut[:], lc[k][:], AFT.Tanh)
                    nc.vector.tensor_tensor(lh[k][:], ot[:], ut[:],
                                            AluOpType.mult)
                emit_hsum(D - 1, lh[k][:], k * (SL // BR), SL // BR,
                          hsum_eng)

            # ---- internal blocks ----
            def make_state(lv, c0, Sp, ch):
                """ch: list of (h_ap, c_ap) child chunk APs, each
                [P, 2, BR*Sp//len(ch)] wide."""
                xt = load_x(lv, c0, Sp, tag="xi", bufs=2)
                fcs = work.tile([P, 2, Sp], BF16, tag="fcs", bufs=2,
                                name="fcs")
                return {"lv": lv, "c0": c0, "Sp": Sp, "xt": xt, "ch": ch,
                        "fcs": fcs, "hs": hs_t[lv][:, :, c0:c0 + Sp],
                        "h_dst": lt_h[lv][:, :, c0:c0 + Sp],
                        "c_dst": lt_c[lv][:, :, c0:c0 + Sp]}

            def int_f_quarter(st, cc, ftt):
                """f-gates for the parents whose children live in child
                chunk cc, feature tile ftt."""
                ncc = len(st["ch"])
                Sq = st["Sp"] // ncc          # parents in this slice
                nq = Sq * BR                  # children
                ch_h, ch_c = st["ch"][cc]
                xt = st["xt"]
                woff = 768 + ftt * P
                pf = psum.tile([P, nq], F32, tag="g", bufs=4, name="pf")
                for q in range(max(1, nq // 512)):
                    w_ = min(512, nq)
                    lo = q * 512
                    dst = pf[:, lo:lo + w_]
                    nc.tensor.matmul(dst, wht[:, 0, woff:woff + P],
                                     ch_h[:, 0, lo:lo + w_],
                                     start=True, stop=False)
                    nc.tensor.matmul(dst, wht[:, 1, woff:woff + P],
                                     ch_h[:, 1, lo:lo + w_],
                                     start=False, stop=False)
                    plo, pw = cc * Sq + lo // BR, w_ // BR
                    for kt in range(2):
                        rhs = xt[:, kt, plo:plo + pw] \
                            .rearrange("p (n b) -> p n b", b=1) \
                            .broadcast_to([P, pw, BR])
                        nc.tensor.matmul(
                            dst.rearrange("p (n b) -> p n b", b=BR),
                            wxt[:, kt, woff:woff + P],
                            rhs, start=False, stop=(kt == 1))
                fq = work.tile([P, nq], BF16, tag="fq", bufs=2, name="fq")
                nc.scalar.activation(fq[:], pf[:], AFT.Sigmoid,
                                     bias=bt[:, 6 + ftt:7 + ftt])
                with nc.allow_low_precision(reason="bf16 by design"):
                    nc.vector.tensor_tensor(fq[:], fq[:], ch_c[:, ftt, :],
                                            AluOpType.mult)
                    v = fq[:].rearrange("p (n b) -> p n b", b=BR)
                    ftmp = work.tile([P, Sq, 2], BF16, tag="ftmp", bufs=2,
                                     name="ftmp")
                    nc.vector.tensor_tensor(ftmp[:], v[:, :, 0:2],
                                            v[:, :, 2:4], AluOpType.add)
                    nc.vector.tensor_tensor(
                        st["fcs"][:, ftt, cc * Sq:(cc + 1) * Sq],
                        ftmp[:, :, 0], ftmp[:, :, 1], AluOpType.add)

            def int_iou_fin(st):
                it, ot, ut = iou_gates(st["xt"], st["Sp"], st["hs"])
                fcs = st["fcs"]
                lv = st["lv"]
                with nc.allow_low_precision(reason="bf16 by design"):
                    nc.vector.tensor_tensor(it[:], it[:], ut[:],
                                            AluOpType.mult)
                    nc.vector.tensor_tensor(st["c_dst"], it[:], fcs[:],
                                            AluOpType.add)
                    nc.scalar.activation(ut[:], st["c_dst"], AFT.Tanh)
                    nc.vector.tensor_tensor(st["h_dst"], ot[:], ut[:],
                                            AluOpType.mult)
                if lv - 1 >= cut:
                    emit_hsum(lv - 1, st["h_dst"], st["c0"] // BR,
                              st["Sp"] // BR, nc.vector)
                if lv == cut:
                    c0, Sp = st["c0"], st["Sp"]
                    nc.sync.dma_start(
                        out_h[:, :, c0:c0 + Sp].rearrange("a p n -> p a n"),
                        st["h_dst"])
                    nc.sync.dma_start(
                        out_c[:, :, c0:c0 + Sp].rearrange("a p n -> p a n"),
                        st["c_dst"])

            # ---- schedule: leaves + level D-1 blocks, pipelined ----
            lp = D - 1
            nleaf = nloc[D] // SL                 # 8 leaf chunks
            nblk = nloc[lp] // SP                 # 4 parent blocks
            blocks = {}
            todo = []          # (fn, args) queue of internal slices
            for k in range(nleaf):
                leaf_chunk(k, nc.gpsimd if k % 2 else nc.vector)
                # drain up to 3 pending internal slices between leaf chunks
                emitted = 0
                while todo and emitted < 3:
                    fn, a = todo.pop(0)
                    fn(*a)
                    emitted += 1
                j = k // 2
                if k % 2 == 0:
                    # children chunk cc=0 ready: start this block's f-gates
                    blocks[j] = make_state(lp, j * SP, SP,
                                           [(lh[k][:], lc[k][:]), None])
                    todo += [(int_f_quarter, (blocks[j], 0, 0)),
                             (int_f_quarter, (blocks[j], 0, 1))]
                else:
                    blocks[j]["ch"][1] = (lh[k][:], lc[k][:])
                    todo += [(int_f_quarter, (blocks[j], 1, 0)),
                             (int_f_quarter, (blocks[j], 1, 1)),
                             (int_iou_fin, (blocks[j],))]
            for fn, a in todo:
                fn(*a)

            # ---- levels D-2 .. cut (children from lt tiles) ----
            for lv in range(D - 2, cut - 1, -1):
                Sp = min(SP, nloc[lv])
                for c0 in range(0, nloc[lv], Sp):
                    nch = BR * Sp
                    ch = [(lt_h[lv + 1][:, :, c0 * BR + i * nch // 2:
                                        c0 * BR + (i + 1) * nch // 2],
                           lt_c[lv + 1][:, :, c0 * BR + i * nch // 2:
                                        c0 * BR + (i + 1) * nch // 2])
                          for i in range(2)]
                    st = make_state(lv, c0, Sp, ch)
                    for cc in range(2):
                        for ftt in range(2):
                            int_f_quarter(st, cc, ftt)
                    int_iou_fin(st)

    nc.compile()
    return nc


def shard_inputs(x, W_iou_x, b_iou_x, W_iou_h, b_iou_h, W_fx, b_fx, W_fh, b_fh,
                 D, cut):
    offs = level_offs(D)
    nloc = local_counts(D, cut)
    wx_cat = np.concatenate([W_iou_x, W_fx], axis=0)
    wh_cat = np.concatenate([W_iou_h, W_fh], axis=0)
    wx_d = np.ascontiguousarray(wx_cat.T).reshape(2, P, 1024).astype(ml_dtypes.bfloat16)
    wh_d = np.ascontiguousarray(wh_cat.T).reshape(2, P, 1024).astype(ml_dtypes.bfloat16)
    b_iou = (b_iou_x + b_iou_h).reshape(6, P).T
    b_f = (b_fx + b_fh).reshape(2, P).T
    bias = np.ascontiguousarray(
        np.concatenate([b_iou, b_f], axis=1)).astype(np.float32)
    in_maps = []
    for k in range(NCORES):
        rows = []
        for l in range(cut, D + 1):
            n = nloc[l]
            rows.append(x[offs[l] + k * n: offs[l] + (k + 1) * n])
        xl = np.concatenate(rows, axis=0)
        xTk = np.ascontiguousarray(xl.T).reshape(2, P, -1).astype(ml_dtypes.bfloat16)
        in_maps.append({"xT": xTk, "wx": wx_d, "wh": wh_d, "bias": bias})
    return in_maps


def finish_host(results, x, W_iou_x, b_iou_x, W_iou_h, b_iou_h,
                W_fx, b_fx, W_fh, b_fh, D, cut):
    ncut = BR ** cut
    npc = ncut // NCORES
    Hc = np.empty((ncut, 256), np.float32)
    Cc = np.empty((ncut, 256), np.float32)
    for k in range(NCORES):
        oh = results[k]["out_h"].astype(np.float32).reshape(256, npc)
        oc = results[k]["out_c"].astype(np.float32).reshape(256, npc)
        Hc[k * npc:(k + 1) * npc] = oh.T
        Cc[k * npc:(k + 1) * npc] = oc.T
    sig = lambda v: 1.0 / (1.0 + np.exp(-v))
    h_next, c_next = Hc, Cc
    for l in range(cut - 1, -1, -1):
        n, off = BR ** l, (BR ** l - 1) // 3
        xl = x[off:off + n]
        child_h = h_next.reshape(n, BR, 256)
        child_c = c_next.reshape(n, BR, 256)
        chs = child_h.sum(axis=1)
        iou = xl @ W_iou_x.T + b_iou_x + chs @ W_iou_h.T + b_iou_h
        i, o, u = np.split(iou, 3, axis=1)
        i, o, u = sig(i), sig(o), np.tanh(u)
        f = sig(child_h @ W_fh.T + b_fh + (xl @ W_fx.T + b_fx)[:, None, :])
        c = i * u + (f * child_c).sum(axis=1)
        h = o * np.tanh(c)
        h_next, c_next = h, c
    return c_next.astype(np.float32), h_next.astype(np.float32)


# ---------------- public API ----------------

_D = 8
_CUT = 7
_CACHE = {}


def _get_program():
    if "nc" not in _CACHE:
        _CACHE["nc"] = build_program(_D, _CUT)
    return _CACHE["nc"]


def kernel(x, W_iou_x, b_iou_x, W_iou_h, b_iou_h, W_fx, b_fx, W_fh, b_fh):
    from concourse import bass_utils
    x = np.asarray(x, dtype=np.float32)
    args = [np.asarray(a, dtype=np.float32) for a in
            (W_iou_x, b_iou_x, W_iou_h, b_iou_h, W_fx, b_fx, W_fh, b_fh)]
    nc = _get_program()
    in_maps = shard_inputs(x, *args, _D, _CUT)
    res = bass_utils.run_bass_kernel_spmd(nc, in_maps,
                                          core_ids=list(range(NCORES)))
    c, h = finish_host(res.results, x, *args, _D, _CUT)
    return c, h
